# revision 20
# baseline (speedup 1.0000x reference)
"""Trainium2 Bass kernel for nn_ContextualViewModel_48833778155979.

Computation (see reference):
    station_feats = x[sx, sy]            # (K, F) gather -- on host (hint: replicate)
    y = station_feats @ W                # (K, F) tiny matmul (0.05% of FLOPs) -- host
    res[h, w, :] = sum_k d[h, w, k] * y[k, :]   # big (H*W, K) @ (K, F) matmul -- device

Sharding: H axis split across 8 cores (48 rows each -> 18432 grid cells/core).
Per core the big matmul is (18432, 256) @ (256, 256).

The kernel is HBM-DMA bound: 16 DMA engines/core at ~26.5 B/ns each
(~400 GB/s). All device I/O is fp16 (halves bytes vs fp32; adds ~5e-4
rel err, far under the 1e-2 gate): d is pre-transposed on the host into
a k-major layout with 8 KiB/partition DMA lines; the output is stored
fp16 f-major and transposed/upcast on the host.

Device structure per core (9 slabs of 2048 grid cells, fully prefetched
into SBUF so the DMA engines never wait on buffer recycling):
  - the tiny y is the matmul STATIONARY operand (4 distinct 128x128
    tiles) and d streams as the MOVING operand at N=512, which halves
    the matmul/ldweights instruction count vs d-stationary and keeps
    the tensor engine (the mid-run pacer) ahead of the DMA stream;
  - PSUM [128, 512] accumulates over the two 128-wide k chunks, then
    one copy per bank (alternating vector/scalar engines) casts to an
    fp16 [128 f, 2 fc, 2048 r] staging tile;
  - slab output DMA (trigger on the otherwise-idle gpsimd engine) with
    4 KiB/partition lines;
  - slab 0's input is split into four 512-row DMAs so the first matmul
    starts ~2.5us earlier and the PE ramps while the rest streams.
"""

import sys

sys.path.insert(0, "/opt/trn_rl_repo")

from contextlib import ExitStack

import numpy as np

import concourse.bacc as bacc
import concourse.mybir as mybir
import concourse.tile as tile
from concourse.bass_utils import run_bass_kernel_spmd

H, WG, F = 384, 384, 256
K = 256
NCORES = 8
HS = H // NCORES          # 48 grid rows per core
ROWS = HS * WG            # 18432 cells per core
RS = 2048                 # rows per DMA slab
NSLAB = ROWS // RS        # 9
GR = 512                  # rows per matmul group (= one PSUM bank of fp32,
                          # and the ISA max moving size per matmul)
NG = RS // GR             # 4 groups per slab
# input DMA piece count per slab: slab 0 in quarters (earliest compute
# start), last slab in halves (shorter serial tail), 8 KiB lines otherwise
PIECES = [4] + [1] * (NSLAB - 2) + [2]

F16 = mybir.dt.float16
F32 = mybir.dt.float32

_cache: dict = {}
last_results = None  # BassKernelResults of the most recent kernel() call


def _build_program(reps: int = 1):
    key = ("nc", reps)
    if key in _cache:
        return _cache[key]

    nc = bacc.Bacc(
        "TRN2", target_bir_lowering=False, debug=False, num_devices=NCORES
    )

    # d, pre-transposed on host: row s*128+kp holds the 2*RS fp16 values
    # [kc, r] = d[s*RS + r, kc*128 + kp]
    d_ext = nc.dram_tensor("d_t", [NSLAB * 128, 2 * RS], F16, kind="ExternalInput").ap()
    # y packed host-side as [kp, kc, f] so each partition is one 1 KiB line
    y_ext = nc.dram_tensor("y_mat", [128, 2 * F], F16, kind="ExternalInput").ap()
    # output f-major: out[f, s*RS + r] (host transposes back)
    out_ext = nc.dram_tensor("out_shard", [F, ROWS], F16, kind="ExternalOutput").ap()

    with tile.TileContext(nc) as tc, ExitStack() as ctx:
        const = ctx.enter_context(tc.tile_pool(name="const", bufs=1))
        dpool = ctx.enter_context(tc.tile_pool(name="din", bufs=NSLAB))
        ppool = ctx.enter_context(tc.tile_pool(name="dpair", bufs=3))
        lpool = ctx.enter_context(tc.tile_pool(name="dlast", bufs=1))
        mpsum = ctx.enter_context(tc.tile_pool(name="mpsum", bufs=6, space="PSUM"))

        din_tiles = {}

        # Front-load the entire input stream: y (tiny, needed first) then
        # slab 0 in four 512-row pieces (so the first matmul starts as
        # early as possible), then the remaining slabs; the last slab in
        # two 1024-row pieces to shorten the serial tail.
        y_sb = const.tile([128, 2, F], F16)
        nc.sync.dma_start(
            y_sb[:, :, :], y_ext.rearrange("p (kc f) -> p kc f", kc=2)
        )

        # Host packs each slab's columns as [piece, kc, r] so every piece
        # DMA is one contiguous chunk per partition (2 KiB for slab-0
        # quarters, 4 KiB for last-slab halves, 8 KiB for whole slabs).
        def issue_din(s, pieces):
            t = dpool.tile([128, 2, RS], F16, tag="din")
            pr = RS // pieces
            for q in range(pieces):
                src = d_ext[
                    s * 128 : (s + 1) * 128, q * 2 * pr : (q + 1) * 2 * pr
                ].rearrange("p (kc r) -> p kc r", kc=2)
                nc.sync.dma_start(t[:, :, q * pr : (q + 1) * pr], src)
            din_tiles[s] = t

        for s in range(NSLAB):
            issue_din(s, PIECES[s])

        # Output staging: slabs 0-7 in PAIRS (one [128, 2, 2*RS] tile per
        # pair -> 8 KiB DMA lines, matching the 8 KiB input packets so the
        # per-packet round-robin between the in/out queues splits engine
        # bandwidth evenly); last slab in two halves for tail overlap.
        pair_tiles = {}

        def emit_slab(s):
            din = din_tiles.pop(s)
            last = s == NSLAB - 1
            if last:
                dout = lpool.tile([128, 2, RS], F16, tag="dlast")
                off = 0
            elif s % 2 == 0:
                dout = ppool.tile([128, 2, 2 * RS], F16, tag="dpair")
                pair_tiles[s // 2] = dout
                off = 0
            else:
                dout = pair_tiles.pop(s // 2)
                off = RS
            for g in range(NG):
                for fc in range(2):
                    po = mpsum.tile([128, GR], F32, tag="po")
                    for kc in range(2):
                        nc.tensor.matmul(
                            po[:, :],
                            y_sb[:, kc, fc * 128 : (fc + 1) * 128],
                            din[:, kc, g * GR : (g + 1) * GR],
                            start=(kc == 0),
                            stop=(kc == 1),
                        )
                    if (g + fc) % 2 == 0:
                        nc.vector.tensor_copy(
                            dout[:, fc, off + g * GR : off + (g + 1) * GR], po[:, :]
                        )
                    else:
                        nc.scalar.copy(
                            dout[:, fc, off + g * GR : off + (g + 1) * GR], po[:, :]
                        )
            if last:
                out_src = out_ext[:, s * RS : (s + 1) * RS].rearrange(
                    "(fc fp) r -> fp fc r", fc=2
                )
                hr = RS // 2
                for h in range(2):
                    nc.gpsimd.dma_start(
                        out_src[:, :, h * hr : (h + 1) * hr],
                        dout[:, :, h * hr : (h + 1) * hr],
                    )
            elif s % 2 == 1:
                nc.gpsimd.dma_start(
                    out_ext[:, (s - 1) * RS : (s + 1) * RS].rearrange(
                        "(fc fp) r -> fp fc r", fc=2
                    ),
                    dout[:, :, :],
                )

        def emit_pipeline():
            for s in range(NSLAB):
                emit_slab(s)

        if reps == 1:
            emit_pipeline()
        else:
            with tc.For_i(0, reps, 1):
                emit_pipeline()

    nc.compile()
    _cache[key] = nc
    return nc


def kernel(x, d, W, sx, sy):
    x = np.asarray(x, dtype=np.float32)
    d = np.asarray(d, dtype=np.float32)
    W = np.asarray(W, dtype=np.float32)
    sx = np.asarray(sx, dtype=np.int32)
    sy = np.asarray(sy, dtype=np.int32)

    # Host-side gather of the K station feature vectors + the tiny (K,F)@(F,F)
    # matmul (replicated to all cores per the sharding strategy), packed
    # [kp, kc, f] for single-line-per-partition DMA.
    y16 = (x[sx, sy] @ W).astype(np.float16)
    y16 = np.ascontiguousarray(
        y16.reshape(2, 128, F).transpose(1, 0, 2)
    ).reshape(128, 2 * F)

    # Pack d k-major per core: dt[c, s, kp, kc, r] = d[row s*RS + r, kc*128+kp],
    # then per slab reorder columns to [piece, kc, r_piece] to match the
    # device's piece DMAs (one contiguous chunk per partition per piece).
    d16 = d.astype(np.float16)
    dv = d16.reshape(NCORES, NSLAB, RS, 2, 128)  # [c, s, r, kc, kp]
    dt0 = dv.transpose(0, 1, 4, 3, 2)            # [c, s, kp, kc, r]
    dt = np.empty((NCORES, NSLAB, 128, 2 * RS), dtype=np.float16)
    for s, P in enumerate(PIECES):
        blk = dt0[:, s].reshape(NCORES, 128, 2, P, RS // P)
        dt[:, s] = blk.transpose(0, 1, 3, 2, 4).reshape(NCORES, 128, 2 * RS)

    nc = _build_program()

    in_maps = []
    for c in range(NCORES):
        in_maps.append(
            {
                "d_t": dt[c].reshape(NSLAB * 128, 2 * RS),
                "y_mat": y16,
            }
        )

    res = run_bass_kernel_spmd(nc, in_maps, list(range(NCORES)))
    global last_results
    last_results = res
    out = np.concatenate(
        [r["out_shard"].T.reshape(HS, WG, F) for r in res.results], axis=0
    ).astype(np.float32)
    return out


if __name__ == "__main__":
    rng = np.random.default_rng(0)
    x = rng.standard_normal((H, WG, F), dtype=np.float32)
    d = rng.random((H, WG, K), dtype=np.float32)
    W = rng.standard_normal((K, F), dtype=np.float32) / np.sqrt(F)
    sx = rng.integers(0, H, size=(K,)).astype(np.int32)
    sy = rng.integers(0, WG, size=(K,)).astype(np.int32)
    out = kernel(x, d, W, sx, sy)
    y = x[sx, sy].astype(np.float64) @ W.astype(np.float64)
    exp = d.reshape(-1, K).astype(np.float64) @ y
    exp = exp.reshape(H, WG, F)
    err = np.linalg.norm(out - exp) / np.linalg.norm(exp)
    print("rel err:", err)
